# revision 9
# baseline (speedup 1.0000x reference)
"""Trainium2 Bass kernel for the NoisyRNN problem — block-Picard scheme (v2).

Math (reference, rescaled g = h/EPS):
    g_{t+1} = g_t P + q_{t+1},  q_{t+1} = tanh(g_t W' + z_t)
    P = I + S,  S = EPS*A,  W' = EPS*W            (||S||,||W'|| ~ 2e-3)
    out = (EPS*g_T) @ D_w^T + D_b

Per block of K=32 steps (d = 0..K-1, true block-start state g0, stale
predictor ghat = g0[i-DELTA] @ P^(DELTA*K)):
    iter1:  Y0[d] = z[d] + ghat (W' + dbar_c S W'),   q1 = tanh(Y0 + bE)
    scan (masked prefix along t per (u,b), ghat injected at slot 0):
       c0[d] = ghat + sum_{j<=d} q1[j-1]
    iter2:  dg = g0 - ghat
            Y1[d] = z[d] + c0[d] W' + ghat (dbar_c S W') + dg (W' + dbar_c S W')
            q2 = tanh(Y1 + bE)
    boundary (pair-merged, exact g0 term):
            g0' = g0 P^K + sum_p (q2[2p] + q2[2p+1]) (P^(K-2p-1)+P^(K-2p-2))/2

Approximations vs exact recurrence (all validated in numpy, total ~2.6e-3
rel err vs 2e-2 tolerance): one Picard correction; chunk-midpoint dbar for
the d-weighted S W' terms; dropped sum_j (d-j) q_j S W' (second order in S);
pair-merged boundary powers; fp16 operands.

Pipeline: iter-1 runs on transient psum chunks (freed right after ACT1) two
blocks ahead of iter-2, so the in-order ACT stream never waits on the scan.
The g0-dependent work per block is only: dg sub, 4 bcast matmuls, ACT2,
boundary, g-copy (short serial chain).

Layouts per block (free dim, per core):
    transient psum (iter1): chunks = b-range [16 bc, 16 bc+16) x K=32 t,
        b-outer — so ACT1 writes qs1 PACKED (offset +1)
    iter2 psum: chunks = t-range [8c, 8c+8) x 64 b, t-outer — so ACT2
        writes qs2 packed in t-outer layout and the boundary pipelines
    x block tile [64, BL*K] b-outer (slot b*K + t): transient z-mm rhs is
        contiguous, iter2 z-mm rhs is the strided (t,b) view
    qs1 [128, BL*K+1] b-outer: slot b*K+(d+1) = q1[d]; slot b*K = ghat
    c0 [128, BL*K] b-outer;  qs2 [128, BL*K] t-outer (slot d*64+b)
"""

import numpy as np

import concourse.bass as bass
import concourse.tile as tile
from concourse.tile import add_dep_helper
from concourse import bacc, mybir
from concourse.bass_utils import run_bass_kernel_spmd
from concourse.ap import AP

EPS = 0.01
BETA = 0.8
GAMMA_A = 0.01
GAMMA_W = 0.01
ALPHA = 1.0
NU = 128
DIN = 64
COUT = 10
B_FULL = 512
T_FULL = 1024
NCORES = 8
BL = B_FULL // NCORES   # 64 batch per core

K = 32                  # timesteps per block
NCH = 4                 # psum chunks per block
TCH = K // NCH          # t per chunk (8)
BCH = BL // NCH         # b per transient chunk (16)
CHW = TCH * BL          # chunk width (512)
MERGE = 8
NGRP = K // MERGE
DELTA = 2               # predictor staleness in blocks

F32 = mybir.dt.float32
F16 = mybir.dt.float16
Tanh = mybir.ActivationFunctionType.Tanh
ADD = mybir.AluOpType.add
SUB = mybir.AluOpType.subtract
MULT = mybir.AluOpType.mult


def build_rnn(T: int, warmup_mms: int = 24) -> bass.Bass:
    nblk = T // K
    assert nblk >= 4
    nc = bacc.Bacc("TRN2", target_bir_lowering=False, debug=False)

    _last_pe = [None]

    import os as _os
    _pin = _os.environ.get("RNN_PIN", "1") == "1"

    def mm(*args, **kwargs):
        # pin PE program order to emission order (RNN_PIN=0 to disable)
        inst = nc.tensor.matmul(*args, **kwargs)
        if _pin:
            cur = getattr(inst, "ins", inst)
            if _last_pe[0] is not None:
                add_dep_helper(cur, _last_pe[0], sync=False,
                               reason="pe-order-pin")
            _last_pe[0] = cur
        return inst

    xw = nc.dram_tensor("xw", [DIN, T * BL], F16, kind="ExternalInput")
    wE = nc.dram_tensor("wE", [DIN, NU], F16, kind="ExternalInput")
    wWp = nc.dram_tensor("wWp", [NU, NU], F16, kind="ExternalInput")
    wCmbG = nc.dram_tensor("wCmbG", [NU, NU], F16, kind="ExternalInput")
    wPD = nc.dram_tensor("wPD", [NU, NU], F16, kind="ExternalInput")
    wPK = nc.dram_tensor("wPK", [NU, NU], F16, kind="ExternalInput")
    wPB = nc.dram_tensor("wPB", [NU, NGRP * NU], F16, kind="ExternalInput")
    wD = nc.dram_tensor("wD", [NU, COUT], F16, kind="ExternalInput")
    bE = nc.dram_tensor("bE", [NU, 1], F32, kind="ExternalInput")
    bD = nc.dram_tensor("bD", [COUT, 1], F32, kind="ExternalInput")
    maskd = nc.dram_tensor("maskd", [NU, BL * K], F16, kind="ExternalInput")
    out = nc.dram_tensor("out", [COUT, BL], F32, kind="ExternalOutput")

    with tile.TileContext(nc) as tc:
        with (
            tc.tile_pool(name="const", bufs=1) as cp,
            tc.tile_pool(name="xp", bufs=5) as xp,
            tc.tile_pool(name="qs1p", bufs=3) as qs1p,
            tc.tile_pool(name="c0p", bufs=3) as c0p,
            tc.tile_pool(name="qs2p", bufs=2) as qs2p,
            tc.tile_pool(name="prp", bufs=2) as prp,
            tc.tile_pool(name="gp", bufs=5) as gp,
            tc.tile_pool(name="ghp", bufs=3) as ghp,
            tc.tile_pool(name="dgp", bufs=2) as dgp,
            tc.tile_pool(name="op_", bufs=1) as op_,
            tc.tile_pool(name="psy", bufs=7, space="PSUM") as psy,
            tc.tile_pool(name="pss", bufs=1, space="PSUM") as pss,
        ):
            # ---- constants ----
            wE_t = cp.tile([DIN, NU], F16, tag="wE")
            nc.sync.dma_start(wE_t[:], wE[:])
            wWp_t = cp.tile([NU, NU], F16, tag="wWp")
            nc.sync.dma_start(wWp_t[:], wWp[:])
            wCmbG_t = cp.tile([NU, NU], F16, tag="wCmbG")
            nc.sync.dma_start(wCmbG_t[:], wCmbG[:])
            wPD_t = cp.tile([NU, NU], F16, tag="wPD")
            nc.sync.dma_start(wPD_t[:], wPD[:])
            wPK_t = cp.tile([NU, NU], F16, tag="wPK")
            nc.sync.dma_start(wPK_t[:], wPK[:])
            wPB_t = cp.tile([NU, NGRP * NU], F16, tag="wPB")
            nc.sync.dma_start(wPB_t[:], wPB[:])
            wD_t = cp.tile([NU, COUT], F16, tag="wD")
            nc.sync.dma_start(wD_t[:], wD[:])
            bE_t = cp.tile([NU, 1], F32, tag="bE")
            nc.sync.dma_start(bE_t[:], bE[:])
            bD_t = cp.tile([COUT, 1], F32, tag="bD")
            nc.sync.dma_start(bD_t[:], bD[:])
            mask_t = cp.tile([NU, BL * K], F16, tag="mask")
            nc.sync.dma_start(mask_t[:], maskd[:])

            zg_t = cp.tile([NU, BL], F16, tag="zg")      # zero ghat/g0
            nc.gpsimd.memset(zg_t[:], 0.0)

            # ---- ACT tanh table preload ----
            scratch = cp.tile([NU, 1], F32, tag="scratch")
            nc.scalar.activation(scratch[:], bE_t[:], Tanh, bias=0.0)

            # ---- PE warmup (lift HAM while DMAs land) ----
            warm = psy.tile([NU, CHW], F32, tag="y", name="warm")
            for _ in range(warmup_mms):
                mm(warm[:], wPK_t[:], mask_t[:, :CHW], start=True, stop=True)

            def pb(p):
                return wPB_t[:, p * NU:(p + 1) * NU]

            psb_t = [None] * nblk           # boundary psum per block
            g_t = [None] * (nblk + 1)       # true g0 per block, fp16 [NU, BL]
            gh_t = [None] * (nblk + 1)      # ghat per block
            x_t = [None] * nblk
            c0_t = [None] * nblk

            g_t[0] = zg_t
            gh_t[0] = zg_t
            gh_t[1] = zg_t

            def fetch_x(i):
                xt = xp.tile([DIN, BL * K], F16, tag="x", name=f"x{i}")
                nc.sync.dma_start(xt[:], xw[:, i * BL * K:(i + 1) * BL * K])
                x_t[i] = xt

            def pred_mm(j):
                """ghat prediction for block j (emitted 2 blocks early)."""
                if j >= DELTA and j < nblk:
                    psp = psy.tile([NU, CHW], F32, tag="y", name=f"pred{j}")
                    mm(psp[:, :BL], wPD_t[:], g_t[j - DELTA][:],
                       start=True, stop=True)
                    gh_t[j] = (psp, None)   # placeholder until ghat_copy

            def ghat_copy(j):
                if j >= DELTA and j < nblk:
                    psp, _ = gh_t[j]
                    gh = ghp.tile([NU, BL], F16, tag="gh", name=f"gh{j}")
                    nc.scalar.copy(gh[:], psp[:, :BL])
                    gh_t[j] = gh

            def close_main(i):
                """iter2 matmuls + ACT2 + copies + group-sums for block i."""
                c0 = c0_t[i]
                if i == 0:
                    dg = None                      # g0 = ghat = 0
                elif i < DELTA:
                    dg = g_t[i]                    # ghat = 0
                else:
                    dg = dgp.tile([NU, BL], F16, tag="dg", name=f"dg{i}")
                    nc.gpsimd.tensor_tensor(dg[:], g_t[i][:], gh_t[i][:], SUB)
                if dg is not None:
                    # fold the true-g0 correction into c0 (bcast over t);
                    # its (d-weighted S W') part is negligible
                    dg_bc = AP(dg.tensor, dg.offset,
                               [list(dg.ap[0]), [1, BL], [0, K]])
                    nc.vector.tensor_tensor(c0[:], c0[:], dg_bc, ADD)
                pred_mm(i + 2)
                ys = []
                for c in range(NCH):
                    y = psy.tile([NU, CHW], F32, tag="y", name=f"y{i}_{c}")
                    ys.append(y)
                    xv = AP(x_t[i].tensor, x_t[i].offset + c * TCH,
                            [list(x_t[i].ap[0]), [1, TCH], [K, BL]])
                    mm(y[:], wE_t[:], xv, start=True, stop=False)
                    c0v = AP(c0.tensor, c0.offset + c * TCH,
                             [list(c0.ap[0]), [1, TCH], [K, BL]])
                    mm(y[:], wWp_t[:], c0v, start=False, stop=True)
                # boundary psum + exact g0 term (early, off the q-path)
                psb = pss.tile([NU, CHW], F32, tag="ps", name=f"bnd{i}")
                psb_t[i] = psb
                if i > 0:
                    mm(psb[:, :BL], wPK_t[:], g_t[i][:], start=True, stop=False)
                qs2 = qs2p.tile([NU, BL * K], F16, tag="qs2", name=f"qs2_{i}")
                for c in range(NCH):
                    nc.scalar.activation(
                        qs2[:, c * CHW:(c + 1) * CHW], ys[c][:],
                        Tanh, bias=bE_t[:])
                ghat_copy(i + 2)
                # oct-sum q2 along t (t-outer: adjacent 64-slices)
                pr1 = prp.tile([NU, (K // 2) * BL], F16, tag="pr1",
                               name=f"pr1_{i}")
                ev = AP(qs2.tensor, qs2.offset,
                        [list(qs2.ap[0]), [2 * BL, K // 2], [1, BL]])
                od = AP(qs2.tensor, qs2.offset + BL,
                        [list(qs2.ap[0]), [2 * BL, K // 2], [1, BL]])
                nc.vector.tensor_tensor(pr1[:], ev, od, ADD)
                pr2 = prp.tile([NU, (K // 4) * BL], F16, tag="pr2",
                               name=f"pr2_{i}")
                ev2 = AP(pr1.tensor, pr1.offset,
                         [list(pr1.ap[0]), [2 * BL, K // 4], [1, BL]])
                od2 = AP(pr1.tensor, pr1.offset + BL,
                         [list(pr1.ap[0]), [2 * BL, K // 4], [1, BL]])
                nc.vector.tensor_tensor(pr2[:], ev2, od2, ADD)
                pr3 = prp.tile([NU, NGRP * BL], F16, tag="pr3", name=f"pr3_{i}")
                ev3 = AP(pr2.tensor, pr2.offset,
                         [list(pr2.ap[0]), [2 * BL, NGRP], [1, BL]])
                od3 = AP(pr2.tensor, pr2.offset + BL,
                         [list(pr2.ap[0]), [2 * BL, NGRP], [1, BL]])
                nc.vector.tensor_tensor(pr3[:], ev3, od3, ADD)
                return pr3

            def front_a(i):
                """Predictor matmuls for block i (stale-state only)."""
                gh = gh_t[i]
                yts = []
                for bc in range(NCH):
                    y = psy.tile([NU, CHW], F32, tag="y", name=f"yt{i}_{bc}")
                    yts.append(y)
                    mm(y[:], wE_t[:], x_t[i][:, bc * CHW:(bc + 1) * CHW],
                       start=True, stop=(i < DELTA))
                    if i >= DELTA:
                        gh_bc = AP(gh.tensor, gh.offset + bc * BCH,
                                   [list(gh.ap[0]), [1, BCH], [0, K]])
                        mm(y[:], wCmbG_t[:], gh_bc, start=False, stop=True)
                return yts

            def close_tail(i, pr3):
                """grouped boundary accumulation for block i."""
                psb = psb_t[i]
                for p in range(NGRP):
                    mm(psb[:, :BL], pb(p), pr3[:, p * BL:(p + 1) * BL],
                       start=(i == 0 and p == 0), stop=(p == NGRP - 1))
                gn = gp.tile([NU, BL], F16, tag="g", name=f"g{i + 1}")
                nc.scalar.copy(gn[:], psb[:, :BL])
                g_t[i + 1] = gn

            def front_b(i, yts):
                """ACT1 + inject + scan for block i."""
                gh = gh_t[i]
                qs1 = qs1p.tile([NU, BL * K + 1], F16, tag="qs1", name=f"qs1_{i}")
                for bc in range(NCH):
                    nc.scalar.activation(
                        qs1[:, bc * CHW + 1:(bc + 1) * CHW + 1], yts[bc][:],
                        Tanh, bias=bE_t[:])
                # inject ghat into slots {b*K} (Pool keeps it off the ACT queue)
                slots = AP(qs1.tensor, qs1.offset, [list(qs1.ap[0]), [K, BL]])
                nc.gpsimd.tensor_tensor(slots, gh[:], zg_t[:], ADD)
                c0 = c0p.tile([NU, BL * K], F16, tag="c0", name=f"c0_{i}")
                nc.vector.tensor_tensor_scan(
                    c0[:], mask_t[:], qs1[:, :BL * K], 0.0, MULT, ADD)
                c0_t[i] = c0

            # ---- software pipeline: front runs 2 blocks ahead ----
            for i in range(4):
                fetch_x(i)
            y0 = front_a(0)
            front_b(0, y0)
            y1 = front_a(1)
            front_b(1, y1)
            for i in range(nblk):
                if i + 4 < nblk:
                    fetch_x(i + 4)
                pr3 = close_main(i)
                if i + 2 < nblk:
                    yn = front_a(i + 2)
                close_tail(i, pr3)
                if i + 2 < nblk:
                    front_b(i + 2, yn)

            # ---- epilogue: project g_T ----
            pso = pss.tile([NU, CHW], F32, tag="ps", name="outp")
            mm(pso[:COUT, :BL], wD_t[:], g_t[nblk][:], start=True, stop=True)
            o_t = op_.tile([COUT, BL], F32, tag="o")
            nc.scalar.add(o_t[:], pso[:COUT, :BL], bD_t[:])
            nc.sync.dma_start(out[:], o_t[:])

    nc.compile()
    return nc


def host_prep(x, E_w, E_b, B_p, C_p, D_w, D_b, T=None):
    """Derived matrices + per-core shards. Returns in_maps list."""
    if T is None:
        T = x.shape[1]
    nblk = T // K
    I = np.eye(NU, dtype=np.float64)
    B_p = B_p.astype(np.float64)
    C_p = C_p.astype(np.float64)
    A = BETA * (B_p - B_p.T) + (1.0 - BETA) * (B_p + B_p.T) - GAMMA_A * I
    W = BETA * (C_p - C_p.T) + (1.0 - BETA) * (C_p + C_p.T) - GAMMA_W * I
    S = (EPS * ALPHA) * A
    P = I + S
    Wp = EPS * W
    SWp = S @ Wp

    Ppow = [np.eye(NU)]
    for _ in range(K):
        Ppow.append(Ppow[-1] @ P)
    PD = np.linalg.matrix_power(P, DELTA * K)
    wCmbG = Wp + ((K - 1) / 2.0) * SWp
    wPB = np.concatenate(
        [sum(Ppow[K - MERGE * p - m - 1] for m in range(MERGE)) / MERGE
         for p in range(NGRP)], axis=1)

    mask = np.ones((NU, BL, K), dtype=np.float16)
    mask[:, :, 0] = 0.0

    common = dict(
        wE=E_w.T.astype(np.float16),
        wWp=Wp.astype(np.float16),
        wCmbG=wCmbG.astype(np.float16),
        wPD=PD.astype(np.float16),
        wPK=Ppow[K].astype(np.float16),
        wPB=wPB.astype(np.float16),
        wD=(EPS * D_w.astype(np.float64)).T.astype(np.float16),
        bE=E_b.reshape(NU, 1).astype(np.float32),
        bD=D_b.reshape(COUT, 1).astype(np.float32),
        maskd=mask.reshape(NU, BL * K),
    )

    nb = x.shape[0] // BL
    in_maps = []
    for i in range(nb):
        xc = x[i * BL:(i + 1) * BL, :T, :]             # [BL, T, DIN]
        # per-block b-outer layout: xw[d, blk*BL*K + b*K + t]
        xpre = np.ascontiguousarray(
            xc.reshape(BL, nblk, K, DIN).transpose(3, 1, 0, 2)
            .reshape(DIN, T * BL)
        ).astype(np.float16)
        in_maps.append(dict(xw=xpre, **common))
    return in_maps


def assemble_out(results):
    return np.concatenate([r["out"].T for r in results], axis=0).astype(np.float32)


def kernel(x, E_w, E_b, B_p, C_p, D_w, D_b):
    x = np.asarray(x, dtype=np.float32)
    E_w = np.asarray(E_w, dtype=np.float32)
    E_b = np.asarray(E_b, dtype=np.float32)
    B_p = np.asarray(B_p, dtype=np.float32)
    C_p = np.asarray(C_p, dtype=np.float32)
    D_w = np.asarray(D_w, dtype=np.float32)
    D_b = np.asarray(D_b, dtype=np.float32)
    nc = build_rnn(T_FULL)
    in_maps = host_prep(x, E_w, E_b, B_p, C_p, D_w, D_b, T=T_FULL)
    res = run_bass_kernel_spmd(nc, in_maps, core_ids=list(range(NCORES)))
    return assemble_out(res.results)
